# revision 10
# baseline (speedup 1.0000x reference)
"""Trainium2 Bass kernel for nn_GCN_45028437131774 (gnn_message_passing).

3-layer GCN (MMGCN-style) over N=100k nodes / E=2M edges, dim 64.

v12 design — descriptor-count-minimal aggregation. Profiling showed the
DMA descriptor rate (~7.6 ns/desc aggregate over 16 engines, independent
of locality/size) is the binding resource; the v2 baseline's padded-CSR
spent 2.81 descriptors per edge. v12 spends ~1.05 + a small merge:

  - Nodes sharded by destination across 8 cores (12500 each, padded to
    12544). Per-layer node table [100352, 128] bf16 (64 feats + 64 junk)
    replicated via AllGather (as v2).
  - Per src-quadrant q (32768-row gather windows, int16 idx): dsts are
    binned by their per-quadrant in-degree g. Degree-class runs use the
    max run length over the 8 cores (SPMD shares one program), ~5% pad.
    Slots (class-major, dst, k<g) are gathered EXACTLY (pads point at
    zero pad-rows of the table); one strided tensor_reduce per class
    piece produces a compact per-quadrant z_q [64, ncols_q] bf16 in
    degree-sorted dst order.
  - z_q is DMA-transposed and written to DRAM (zd01/zd23 hold two
    quadrant sections + zero rows). A second-level merge gather (2 slots
    per dst, k=2 reduce) rebuilds z in natural dst order: pass A merges
    q0+q1 into xz[:64], pass B merges q2+q3 into zB; xz[:64] += zB.
  - xz [128, 12544] bf16: rows 0-63 = z, 64-127 = x. Dense layer = v2's
    (block weights in bf16, ACT bias evac + one-DVE leaky).
  - Table write-back + AllGather per layer as v2.

kernel(**inputs) -> (mu, logvar), both [100000, 64] fp32.
"""

import os
import sys

import numpy as np

for _p in ("/opt/trn_rl_repo",):
    if _p not in sys.path and os.path.isdir(_p):
        sys.path.insert(0, _p)

import concourse.bacc as bacc
import concourse.bass as bass
import concourse.mybir as mybir
import concourse.tile as tile

F32 = mybir.dt.float32
BF16 = mybir.dt.bfloat16
I16 = mybir.dt.int16

QUAD = 32768


class Cfg:
    def __init__(self, n_user=50000, n_item=50000, dim_feat=128, d=64,
                 e=2_000_000, ncores=8, tile=128, maxg=4096, chunk=4096,
                 dc=500, num_layer=3, neg=0.01):
        self.n_user, self.n_item = n_user, n_item
        self.n = n_user + n_item
        self.dim_feat, self.d, self.e = dim_feat, d, e
        self.ncores, self.tile = ncores, tile
        self.maxg, self.chunk, self.dc = maxg, chunk, dc
        self.num_layer, self.neg = num_layer, neg
        assert self.n % ncores == 0
        self.shard = self.n // ncores
        self.nt = (self.shard + tile - 1) // tile
        self.pshard = self.nt * tile
        self.tabn = ncores * self.pshard
        self.nq = (self.tabn + QUAD - 1) // QUAD
        assert self.shard % dc == 0
        self.ndc = self.shard // dc


# ---------------------------------------------------------------- host prep

class Geo:
    pass


class Prep:
    pass


def prep_edges(cfg: Cfg, edge_index: np.ndarray) -> Prep:
    """Cross-core-uniform geometry + per-core idx data."""
    src = edge_index[0].astype(np.int64)
    dst = edge_index[1].astype(np.int64)
    prow = (src // cfg.shard) * cfg.pshard + (src % cfg.shard)
    q_of = prow // QUAD
    qi_of = prow % QUAD
    c_of = dst // cfg.shard
    ld_of = dst % cfg.shard

    # zero pad-row inside each quadrant window (table pad rows are zero)
    zrow = []
    for q in range(cfg.nq):
        r = None
        for c in range(cfg.ncores):
            cand = c * cfg.pshard + cfg.shard
            if q * QUAD <= cand < min((q + 1) * QUAD, cfg.tabn):
                r = cand - q * QUAD
                break
        assert r is not None, f"no zero row for quadrant {q}"
        zrow.append(r)

    p = Prep()
    p.geos = []
    gidx_all = [[] for _ in range(cfg.ncores)]
    rank = np.full((cfg.nq, cfg.ncores, cfg.shard), -1, dtype=np.int64)
    for q in range(cfg.nq):
        deg = np.zeros((cfg.ncores, cfg.shard), dtype=np.int64)
        m = q_of == q
        np.add.at(deg, (c_of[m], ld_of[m]), 1)
        gmax = int(deg.max())
        cnt = np.zeros((cfg.ncores, gmax + 1), dtype=np.int64)
        for c in range(cfg.ncores):
            cnt[c] = np.bincount(deg[c], minlength=gmax + 1)
        n_g = cnt.max(axis=0)
        geo = Geo()
        geo.q = q
        geo.zrow = zrow[q]
        geo.runs = [(g, int(n_g[g])) for g in range(1, gmax + 1)
                    if n_g[g] > 0]
        geo.nslot_raw = int(sum(g * n for g, n in geo.runs))
        geo.nslot = -(-geo.nslot_raw // 128) * 128
        geo.ncols = int(sum(n for _, n in geo.runs))
        geo.ncp = -(-geo.ncols // 128) * 128
        p.geos.append(geo)

        for c in range(cfg.ncores):
            gi = np.full(geo.nslot, zrow[q], dtype=np.int16)
            sel = np.nonzero(m & (c_of == c))[0]
            ldc = ld_of[sel]
            qic = qi_of[sel]
            order = np.argsort(ldc, kind="stable")
            ldc, qic = ldc[order], qic[order]
            starts = np.r_[0, np.nonzero(np.diff(ldc))[0] + 1]
            glen = np.diff(np.r_[starts, len(ldc)])
            krank = np.arange(len(ldc)) - np.repeat(starts, glen)
            dd = deg[c]
            slot0 = col0 = 0
            for g, n in geo.runs:
                ds = np.nonzero(dd == g)[0]
                pos_of_d = np.full(cfg.shard, -1, dtype=np.int64)
                pos_of_d[ds] = np.arange(len(ds))
                emask = dd[ldc] == g
                gi[slot0 + pos_of_d[ldc[emask]] * g + krank[emask]] = \
                    qic[emask].astype(np.int16)
                rank[q, c, ds] = col0 + np.arange(len(ds))
                slot0 += g * n
                col0 += n
            gidx_all[c].append(gi)

    # chunk/piece schedule (uniform across cores). Chunks are gather
    # windows: start %128 (idx wrap + num_idxs%128), span %128, span <=
    # cfg.chunk. Reduce pieces split at dst boundaries; a chunk boundary
    # inside a run re-reads <=127+g slots redundantly (next chunk starts
    # at rounddown of the piece start).
    p.chunks = [[] for _ in range(cfg.nq)]  # per q: (gstart, gspan, pieces)
    for q, geo in enumerate(p.geos):
        # dst-level piece stream: (slot_abs, col, g) per maximal run piece
        stream = []
        slot0 = col0 = 0
        for g, n in geo.runs:
            stream.append((slot0, col0, n, g))
            slot0 += g * n
            col0 += n
        total_raw = slot0
        cur = 0          # chunk gather start (%128)
        pieces = []
        hi = 0           # last covered slot
        si = 0
        done_d = 0
        while si < len(stream):
            s0, c0, n, g = stream[si]
            start = s0 + done_d * g
            # dsts fitting in the current chunk window
            nd = min(n - done_d, (cur + cfg.chunk - start) // g)
            if nd > 0:
                pieces.append((start - cur, c0 + done_d, nd, g))
                done_d += nd
                hi = start + nd * g
                if done_d == n:
                    si += 1
                    done_d = 0
                continue
            # close chunk
            span = -(-(hi - cur) // 128) * 128
            p.chunks[q].append((cur, span, pieces))
            cur = ((s0 + done_d * g) // 128) * 128
            pieces = []
        if pieces:
            span = -(-(hi - cur) // 128) * 128
            p.chunks[q].append((cur, span, pieces))
        # gather overrun margin: spans may reach past total_raw
        need = max((c + s for c, s, _ in p.chunks[q]), default=0)
        geo.nslot = max(geo.nslot, -(-need // 128) * 128)

    # gather idx tensors per core (concatenated quadrants)
    p.goff = []
    off = 0
    for geo in p.geos:
        p.goff.append(off)
        off += geo.nslot
    p.gtot = off
    p.gidx = np.zeros((cfg.ncores, 128, p.gtot // 16), dtype=np.int16)
    for c in range(cfg.ncores):
        arr = np.concatenate(gidx_all[c])
        p.gidx[c] = np.tile(arr.reshape(-1, 16).T, (8, 1))

    # merge idx: pass A (q0,q1) and pass B (q2,q3); 2 slots per dst col
    # zd01 rows: [ncp0 | zero | ncp1 | zero]; same for zd23
    p.mrows = []
    p.midx = np.zeros((cfg.ncores, 128, (2 * 2 * cfg.pshard) // 16),
                      dtype=np.int16)
    for pa, (qa, qb) in enumerate(((0, 1), (2, 3))):
        ga, gb = p.geos[qa], p.geos[qb]
        za, zb = ga.ncp, ga.ncp + 1 + gb.ncp
        p.mrows.append((ga.ncp, gb.ncp, za, zb))
        for c in range(cfg.ncores):
            sl = np.empty((cfg.pshard, 2), dtype=np.int16)
            ra = rank[qa, c]
            rb = rank[qb, c]
            a = np.where(ra >= 0, ra, za)
            b = np.where(rb >= 0, (ga.ncp + 1) + rb, zb)
            sl[:cfg.shard, 0] = a.astype(np.int16)
            sl[:cfg.shard, 1] = b.astype(np.int16)
            sl[cfg.shard:, 0] = za
            sl[cfg.shard:, 1] = zb
            flat = sl.reshape(-1)
            p.midx[c][:, pa * (2 * cfg.pshard) // 16:
                      (pa + 1) * (2 * cfg.pshard) // 16] = \
                np.tile(flat.reshape(-1, 16).T, (8, 1))
    return p


def prep_nodes(cfg: Cfg, features, preference, mlp_w, mlp_b):
    raws = np.zeros((cfg.ncores, cfg.dim_feat, cfg.shard), dtype=np.float32)
    projs = np.zeros((cfg.ncores, cfg.dim_feat, cfg.d), dtype=np.float32)
    biases = np.zeros((cfg.ncores, 128, cfg.d), dtype=np.float32)
    for c in range(cfg.ncores):
        lo, hi = c * cfg.shard, (c + 1) * cfg.shard
        raw = np.zeros((cfg.shard, cfg.dim_feat), dtype=np.float32)
        if hi <= cfg.n_user:
            raw[:, :cfg.d] = preference[lo:hi]
            projs[c][:cfg.d, :] = np.eye(cfg.d, dtype=np.float32)
        elif lo >= cfg.n_user:
            raw[:] = features[lo - cfg.n_user:hi - cfg.n_user]
            projs[c] = mlp_w.T
            biases[c][:] = mlp_b[None, :]
        else:
            raise AssertionError("shard straddles user/item boundary")
        raws[c] = raw.T
    return raws, projs, biases


# ---------------------------------------------------------------- builder

def build_program(cfg: Cfg, p: Prep, reps: int = 1):
    nc = bacc.Bacc("TRN2", target_bir_lowering=False, debug=False)
    d, df = cfg.d, cfg.dim_feat
    nlay = cfg.num_layer

    raw_d = nc.dram_tensor("raw", [df, cfg.shard], F32, kind="ExternalInput")
    proj_d = nc.dram_tensor("proj", [df, d], F32, kind="ExternalInput")
    bias0_d = nc.dram_tensor("bias0", [128, d], F32, kind="ExternalInput")
    ident_d = nc.dram_tensor("ident", [128, 128], F32, kind="ExternalInput")
    w2f_d = [nc.dram_tensor(f"w2f{i}", [2 * d, 2 * d], BF16,
                            kind="ExternalInput") for i in range(nlay)]
    gidx_d = nc.dram_tensor("gidx", [128, p.gtot // 16], I16,
                            kind="ExternalInput")
    midx_d = nc.dram_tensor("midx", [128, (4 * cfg.pshard) // 16], I16,
                            kind="ExternalInput")
    w1_d = [nc.dram_tensor(f"w1c{i}", [2 * d, 2 * d], BF16,
                           kind="ExternalInput") for i in range(nlay + 2)]
    w2_d = [nc.dram_tensor(f"w2c{i}", [2 * d, d], BF16, kind="ExternalInput")
            for i in range(nlay + 2)]
    b1_d = [nc.dram_tensor(f"b1p{i}", [2 * d, 1], F32, kind="ExternalInput")
            for i in range(nlay + 2)]
    gb_d = [nc.dram_tensor(f"gbp{i}", [2 * d, 1], F32, kind="ExternalInput")
            for i in range(nlay + 2)]
    mu_d = nc.dram_tensor("mu_fm", [d, cfg.shard], F32, kind="ExternalOutput")
    lv_d = nc.dram_tensor("lv_fm", [d, cfg.shard], F32, kind="ExternalOutput")

    xshard_d = [nc.dram_tensor(f"xshard{i}", [cfg.pshard, 128], BF16)
                for i in range(2)]
    table_d = [nc.dram_tensor(f"table{i}", [cfg.tabn, 128], BF16,
                              addr_space="Shared") for i in range(2)]
    (ncp0, ncp1, _, _), (ncp2, ncp3, _, _) = p.mrows
    zd_rows = [ncp0 + 1 + ncp1 + 1, ncp2 + 1 + ncp3 + 1]
    zd_d = [nc.dram_tensor(f"zd{i}", [zd_rows[i], 128], BF16)
            for i in range(2)]
    rg = [list(range(cfg.ncores))]

    ID = mybir.ActivationFunctionType.Identity
    SQRT = mybir.ActivationFunctionType.Sqrt
    MUL = mybir.AluOpType.mult
    MAX = mybir.AluOpType.max
    ADD = mybir.AluOpType.add
    AX = mybir.AxisListType.X

    ncp_max = max(g.ncp for g in p.geos)

    with tile.TileContext(nc) as tc, \
            tc.tile_pool(name="const", bufs=1) as const, \
            tc.tile_pool(name="big", bufs=1) as big, \
            tc.tile_pool(name="bg", bufs=2) as bpool, \
            tc.tile_pool(name="idxp", bufs=2) as ipool, \
            tc.tile_pool(name="tmp", bufs=2) as tpool, \
            tc.tile_pool(name="hx", bufs=3) as hxpool, \
            tc.tile_pool(name="sm", bufs=3) as smpool, \
            tc.tile_pool(name="outp", bufs=3) as opool, \
            tc.tile_pool(name="raws", bufs=3) as rpool, \
            tc.tile_pool(name="psA", bufs=2,
                         space=bass.MemorySpace.PSUM) as psA, \
            tc.tile_pool(name="psB", bufs=2,
                         space=bass.MemorySpace.PSUM) as psB:

        from concourse import library_config
        nc.gpsimd.load_library(library_config.attnmlp)
        dma_sem = nc.alloc_semaphore("gsem")

        def load_const(dram, shape, dt=F32):
            t = const.tile(shape, dt, tag=dram.name, name=dram.name + "_s")
            nc.sync.dma_start(t[:], dram[:])
            return t

        proj_s = load_const(proj_d, [df, d])
        bias0_s = load_const(bias0_d, [128, d])
        ident_s = load_const(ident_d, [128, 128])
        w2f_s = [load_const(x, [2 * d, 2 * d], BF16) for x in w2f_d]
        w1_s = [load_const(x, [2 * d, 2 * d], BF16) for x in w1_d]
        w2_s = [load_const(x, [2 * d, d], BF16) for x in w2_d]
        b1_s = [load_const(x, [2 * d, 1]) for x in b1_d]
        gb_s = [load_const(x, [2 * d, 1]) for x in gb_d]
        midx_s = load_const(midx_d, [128, (4 * cfg.pshard) // 16], I16)

        xz = big.tile([128, cfg.pshard], BF16, tag="xz", name="xz")
        zB = big.tile([64, cfg.pshard], BF16, tag="zB", name="zB")
        zq = big.tile([64, ncp_max], BF16, tag="zq", name="zq")
        xfm16 = big.tile([64, cfg.pshard], BF16, tag="xfm16", name="xfm16")
        xnode16 = big.tile([128, cfg.nt * d], BF16, tag="xn16", name="xn16")
        if cfg.pshard > cfg.shard:
            nc.vector.memset(xfm16[:, cfg.shard:], 0.0)
        z16 = big.tile([128, cfg.nt * d], BF16, tag="z16", name="z16")
        nc.vector.memset(z16[:], 0.0)
        for i in range(2):
            nc.sync.dma_start(
                xshard_d[i].ap().rearrange("(pp t) c -> pp t c",
                                           pp=128)[:, :, d:],
                z16[:].rearrange("p (t e) -> p t e", e=d))
        # zero rows in zd buffers (full 128-wide)
        for i, (na, nb, _, _) in enumerate(p.mrows):
            nc.sync.dma_start(zd_d[i][na:na + 1, :], z16[0:1, 0:128])
            nc.sync.dma_start(zd_d[i][na + 1 + nb:na + 2 + nb, :],
                              z16[0:1, 0:128])

        def leaky(ap):
            nc.vector.scalar_tensor_tensor(ap, ap, cfg.neg, ap, MUL, MAX)

        def push_table(layer):
            xs, tb_ = xshard_d[layer % 2], table_d[layer % 2]
            nc.gpsimd.dma_start(xfm16[:, :cfg.shard], xz[64:, :cfg.shard])
            nc.sync.dma_start(
                xnode16[:].rearrange("p (t e) -> p t e", e=d),
                xfm16[:], transpose=True)
            xs_view = xs.ap().rearrange("(t pp) c -> pp t c",
                                        pp=128)[:, :, :d]
            nc.sync.dma_start(xs_view, xnode16[:].rearrange(
                "p (t e) -> p t e", e=d))
            nc.gpsimd.collective_compute(
                "AllGather", mybir.AluOpType.bypass, replica_groups=rg,
                ins=[xs[:]], outs=[tb_[:]])

        def push_zq(q, geo):
            """zq[:, :geo.ncp] -> zd section (node-major, 64 cols used)."""
            buf, (na, nb, _, _) = (0, p.mrows[0]) if q < 2 else \
                (1, p.mrows[1])
            row0 = 0 if q % 2 == 0 else na + 1
            nrows = geo.ncp
            nt_q = nrows // 128
            zt = tpool.tile([128, (ncp_max // 128) * d], BF16, tag="zt",
                            name=f"zt{q}")
            nc.sync.dma_start(
                zt[:, :nt_q * d].rearrange("p (t e) -> p t e", e=d),
                zq[:, :nrows], transpose=True)
            zv = zd_d[buf][row0:row0 + nrows, :].rearrange(
                "(t pp) c -> pp t c", pp=128)[:, :, :d]
            nc.sync.dma_start(zv, zt[:, :nt_q * d].rearrange(
                "p (t e) -> p t e", e=d))

        def aggregate(table):
            # per-quadrant class gathers + reduces -> zq -> zd
            for q, geo in enumerate(p.geos):
                qlo = q * QUAD
                qrows = min(QUAD, cfg.tabn - qlo)
                for (gstart, gspan, pieces) in p.chunks[q]:
                    bt = bpool.tile([128, 1, cfg.chunk], BF16, tag="bt",
                                    name="bt")
                    base = p.goff[q] + gstart
                    it = ipool.tile([128, cfg.chunk // 16], I16, tag="gi",
                                    name="gi")
                    nc.sync.dma_start(
                        it[:, :gspan // 16],
                        gidx_d[:, base // 16:(base + gspan) // 16])
                    off = 0
                    while off < gspan:
                        sz = min(cfg.maxg, gspan - off)
                        nc.gpsimd.dma_gather(
                            bt[:, :, off:off + sz], table[qlo:qlo + qrows, :],
                            it[:, off // 16:(off + sz) // 16],
                            sz, sz, 128, transpose=True, single_packet=False,
                            prepare_only=True, sem=dma_sem)
                        nc.gpsimd.trigger_dma(count=None)
                        off += sz
                    with nc.allow_low_precision(reason="bf16 z partials"):
                        for (srel, c0, nd, g) in pieces:
                            view = bt[:64, 0, srel:srel + nd * g].rearrange(
                                "p (dd k) -> p dd k", k=g)
                            nc.vector.tensor_reduce(zq[:, c0:c0 + nd], view,
                                                    axis=AX, op=ADD)
                push_zq(q, geo)

            # merge passes: A (q0,q1) -> xz[:64], B (q2,q3) -> zB, add
            for pa in range(2):
                out = xz[:64, :] if pa == 0 else zB[:]
                nslots = 2 * cfg.pshard
                base = pa * nslots
                off = 0
                while off < nslots:
                    sz = min(cfg.maxg, nslots - off)
                    bt = bpool.tile([128, 1, cfg.chunk], BF16, tag="bt",
                                    name="btm")
                    nc.gpsimd.dma_gather(
                        bt[:, :, :sz], zd_d[pa][:, :],
                        midx_s[:, (base + off) // 16:(base + off + sz) // 16],
                        sz, sz, 128, transpose=True, single_packet=False,
                        prepare_only=True, sem=dma_sem)
                    nc.gpsimd.trigger_dma(count=None)
                    view = bt[:64, 0, :sz].rearrange("p (dd k) -> p dd k",
                                                     k=2)
                    with nc.allow_low_precision(reason="bf16 z merge"):
                        nc.vector.tensor_reduce(
                            out[:, off // 2:(off + sz) // 2],
                            view, axis=AX, op=ADD)
                    off += sz
            with nc.allow_low_precision(reason="bf16 z merge add"):
                nc.vector.tensor_add(xz[:64, :], xz[:64, :], zB[:])

        def dense_chunk(li, sl, final, out_ap=None):
            ps1 = psA.tile([128, cfg.dc], F32, tag="ps1", name="ps1")
            nc.tensor.matmul(ps1[:], w1_s[li][:], xz[:, sl])
            s1 = hxpool.tile([128, cfg.dc], BF16, tag="s1", name="s1")
            nc.scalar.activation(s1[:], ps1[:], ID, bias=b1_s[li][:])
            leaky(s1[:])
            ps2 = psB.tile([128, cfg.dc], F32, tag="ps2", name="ps2")
            if final:
                nc.tensor.matmul(ps2[:64, :], w2_s[li][:], s1[:])
                ot = opool.tile([64, cfg.dc], F32, tag="ot", name="ot")
                nc.scalar.activation(ot[:], ps2[:64, :], ID,
                                     bias=gb_s[li][:64, :])
                nc.sync.dma_start(out_ap, ot[:])
            else:
                ps2f = psB.tile([128, cfg.dc], F32, tag="ps2", name="ps2f")
                nc.tensor.matmul(ps2f[:], w2f_s[li][:], s1[:])
                nc.scalar.activation(xz[64:, sl], ps2f[64:, :], ID,
                                     bias=gb_s[li][64:, :])
                leaky(xz[64:, sl])

        # ------------------------------------------------------- main flow
        for _rep in range(reps):
            SQ = mybir.ActivationFunctionType.Square
            for t in range(cfg.nt):
                lo = t * cfg.tile
                pw = min(cfg.tile, cfg.shard - lo)
                rawt = rpool.tile([df, cfg.tile], F32, tag="rawt",
                                  name="rawt")
                nc.sync.dma_start(rawt[:, :pw], raw_d[:, lo:lo + pw])
                ps0 = psA.tile([cfg.tile, d], F32, tag="ps1", name="ps0")
                nc.tensor.matmul(ps0[:pw, :], rawt[:, :pw], proj_s[:])
                xb = hxpool.tile([cfg.tile, d], F32, tag="xb", name="xb")
                nc.vector.tensor_add(xb[:pw, :], ps0[:pw, :],
                                     bias0_s[:pw, :])
                sq = hxpool.tile([cfg.tile, d], F32, tag="sq", name="sq")
                red = smpool.tile([cfg.tile, 1], F32, tag="red", name="red")
                nc.scalar.activation(sq[:pw, :], xb[:pw, :], SQ,
                                     accum_out=red[:pw, :])
                nc.vector.tensor_scalar_max(red[:pw, :], red[:pw, :], 1e-24)
                nr = smpool.tile([cfg.tile, 1], F32, tag="nr", name="nr")
                nc.scalar.activation(nr[:pw, :], red[:pw, :], SQRT)
                inv = smpool.tile([cfg.tile, 1], F32, tag="inv", name="inv")
                nc.vector.reciprocal(inv[:pw, :], nr[:pw, :])
                xn = hxpool.tile([cfg.tile, d], F32, tag="xn", name="xn")
                nc.vector.tensor_scalar_mul(xn[:pw, :], xb[:pw, :],
                                            inv[:pw, :])
                pst = psB.tile([d, cfg.tile], F32, tag="ps2", name="pst")
                nc.tensor.transpose(pst[:, :pw], xn[:pw, :],
                                    ident_s[:pw, :pw])
                st = opool.tile([d, cfg.tile], F32, tag="st", name="st")
                nc.scalar.copy(st[:, :pw], pst[:, :pw])
                nc.gpsimd.dma_start(xz[64:, lo:lo + pw], st[:, :pw])
            push_table(0)
            for li in range(nlay):
                aggregate(table_d[li % 2])
                for j in range(cfg.ndc):
                    sl = slice(j * cfg.dc, (j + 1) * cfg.dc)
                    dense_chunk(li, sl, final=False)
                push_table(li + 1)
            aggregate(table_d[nlay % 2])
            for j in range(cfg.ndc):
                sl = slice(j * cfg.dc, (j + 1) * cfg.dc)
                dense_chunk(nlay, sl, final=True, out_ap=mu_d[:, sl])
                dense_chunk(nlay + 1, sl, final=True, out_ap=lv_d[:, sl])

    nc.compile()
    return nc


# ---------------------------------------------------------------- kernel()

def make_in_maps(cfg, p, raws, projs, biases, conv_w, lin_w, lin_b, g_w, g_b):
    d = cfg.d
    ident = np.eye(128, dtype=np.float32)
    import ml_dtypes
    bf = lambda a: np.ascontiguousarray(a).astype(ml_dtypes.bfloat16)  # noqa
    in_maps = []
    for c in range(cfg.ncores):
        m = dict(raw=np.ascontiguousarray(raws[c]),
                 proj=np.ascontiguousarray(projs[c]),
                 bias0=np.ascontiguousarray(biases[c]),
                 gidx=p.gidx[c], midx=p.midx[c], ident=ident)
        for i in range(cfg.num_layer):
            w2f = np.zeros((2 * d, 2 * d), np.float32)
            w2f[:d, d:] = g_w[i][:, :d].T
            w2f[d:, d:] = g_w[i][:, d:].T
            m[f"w2f{i}"] = bf(w2f)
        for i in range(cfg.num_layer + 2):
            w1 = np.zeros((2 * d, 2 * d), np.float32)
            w1[:d, :d] = conv_w[i]
            w1[d:, d:] = lin_w[i].T
            m[f"w1c{i}"] = bf(w1)
            m[f"w2c{i}"] = bf(
                np.concatenate([g_w[i][:, :d].T, g_w[i][:, d:].T], axis=0))
            b1 = np.zeros((2 * d, 1), np.float32)
            b1[d:, 0] = lin_b[i]
            m[f"b1p{i}"] = b1
            m[f"gbp{i}"] = np.ascontiguousarray(
                np.tile(g_b[i].reshape(-1, 1), (2, 1)))
        in_maps.append(m)
    return in_maps


def kernel(features, edge_index, preference, mlp_w, mlp_b,
           conv_w, lin_w, lin_b, g_w, g_b, cfg: Cfg | None = None,
           _run=None, reps: int = 1):
    cfg = cfg or Cfg()
    features = np.asarray(features, dtype=np.float32)
    edge_index = np.asarray(edge_index)
    preference = np.asarray(preference, dtype=np.float32)
    mlp_w = np.asarray(mlp_w, dtype=np.float32)
    mlp_b = np.asarray(mlp_b, dtype=np.float32)
    conv_w = [np.asarray(w, dtype=np.float32) for w in conv_w]
    lin_w = [np.asarray(w, dtype=np.float32) for w in lin_w]
    lin_b = [np.asarray(w, dtype=np.float32) for w in lin_b]
    g_w = [np.asarray(w, dtype=np.float32) for w in g_w]
    g_b = [np.asarray(w, dtype=np.float32) for w in g_b]

    p = prep_edges(cfg, edge_index)
    raws, projs, biases = prep_nodes(cfg, features, preference, mlp_w, mlp_b)
    nc = build_program(cfg, p, reps=reps)
    in_maps = make_in_maps(cfg, p, raws, projs, biases,
                           conv_w, lin_w, lin_b, g_w, g_b)

    if _run is not None:
        results = _run(nc, in_maps)
    else:
        from concourse.bass_utils import run_bass_kernel_spmd
        global LAST_RESULTS
        LAST_RESULTS = run_bass_kernel_spmd(
            nc, in_maps, list(range(cfg.ncores)))
        results = LAST_RESULTS.results

    mu = np.concatenate([results[c]["mu_fm"] for c in range(cfg.ncores)],
                        axis=1).T.copy()
    lv = np.concatenate([results[c]["lv_fm"] for c in range(cfg.ncores)],
                        axis=1).T.copy()
    return mu, lv
